# revision 69
# baseline (speedup 1.0000x reference)
"""Trainium2 Bass kernel for truncated BCH on 3D vector fields.

Math (matches the jax reference):
  out_i = l_i + r_i + 0.25 * sum_j ( D_j l_i * r_j  -  D_j r_i * l_j )
where D_j v = v[.+1] - v[.-1] along spatial axis j (circulant wrap), for
j in {X, Y, Z}, channels i in {0,1,2}.

Host-side change of variables (free): u = l + r, w = l - r, giving
  out_i = u_i + 0.125 * sum_j ( D_j w_i * u_j  -  D_j u_i * w_j )
so the linear term is just the input u (no on-device l+r), injected via
an 8*I matmul and recovered by the final 0.125 scale.

Sharding: 8 cores = 2 batches x 4 X-slabs of 32 planes (+1 halo plane
each side, wrapped).  Host lays data per core as one combined tensor
uw = (Y, side, ch, X_halo, Z_pad) fp16, side0 = u, side1 = w, so paired
ops can read (u_j, w_j) multiplicand pairs as a single strided AP.

Per-core engine split:
  - TensorE : Y-diffs as circulant shift-difference matmuls (+-DyT),
              plus PSUM accumulation: 8I x u inject, plain fp16 I-matmuls
              for the x-products, and fp8e4m3 DoubleRow matmuls (0.5
              cycles/row) that sum each (w-side, u-side) product pair of
              the y/z terms in one quarter-cost instruction.
  - VectorE : X- and Z-diffs (shifted-AP subtract, sign-folded on the
              u side) and the x-product pair (fp16, keeps DVE 2x mode).
  - GPSIMD  : y/z-product pairs, written directly as fp8e4m3 (GPSIMD has
              no 16-bit speedup to lose).
  - ScalarE : dy PSUM->SBUF fp16 evacuation (batched 8-plane groups) and
              the final 0.125-scaled PSUM->SBUF fp16 evacuation.
Output is fp16 on device; the host upcasts to f32.
"""

import sys

sys.path.insert(0, "/opt/trn_rl_repo")

import numpy as np

import concourse.bass as bass
import concourse.bacc as bacc
import concourse.mybir as mybir
import concourse.tile as tile
from concourse.bass_utils import run_bass_kernel_spmd

B, D, X, Y, Z = 2, 3, 128, 128, 128
NCORES = 8
XS = (B * X) // NCORES  # 32 output x-planes per core
ZP = Z + 4              # z padded: [z126, z127, z0..z127, z0, z1]
KX = 4                  # x-planes per psum acc chunk (bank = 512 f32)
# (kb, kx) work items; kb multiple of kx.  Tiny first item shortens the
# pipeline fill (starts after only 4 input planes land); tiny last item
# shortens the drain tail.
SIZES = [(4, 4), (8, 4), (8, 4), (8, 4), (4, 4)]

F16 = mybir.dt.float16
F32 = mybir.dt.float32
F8 = mybir.dt.float8e4

DR = mybir.MatmulPerfMode.DoubleRow

# Axes whose channel-0 diff pair is precomputed on the host and streamed in
# as an extra input (rides spare DMA capacity, cuts saturated DVE work).
HOST_AXES = ()


def _make_wmats() -> np.ndarray:
    """[DyT | -DyT | I | 8I] as one (Y, 4Y) fp16 matrix (lhsT layout).

    matmul(out, lhsT, rhs) computes lhsT.T @ rhs.  We want Dy @ v with
    Dy[y, y'] = delta(y'=y+1) - delta(y'=y-1) (wrap), so lhsT = Dy.T.
    """
    e = np.eye(Y, dtype=np.float32)
    dy = np.roll(e, -1, axis=0) - np.roll(e, 1, axis=0)
    dyt = dy.T
    mats = np.concatenate([dyt, -dyt, e, 8.0 * e], axis=1)
    return mats.astype(np.float16)


def _make_w8() -> np.ndarray:
    """DoubleRow pair-sum weights: W8[k, t, m] = delta(k, m), t = 0, 1.

    lhsT.T @ rhs with this weight sums the two k-tile slots of the rhs:
    out[m, n] = rhs[m, 0, n] + rhs[m, 1, n].  Identity is exact in fp8.
    """
    e = np.eye(Y, dtype=np.float32)
    w8 = np.stack([e, e], axis=1)  # (Y, 2, Y)
    import ml_dtypes
    return w8.astype(ml_dtypes.float8_e4m3fn)


def build_nc(xs: int = XS, *, dbufs: int = 4, pbufs: int = 4,
             accbufs: int = 2, sbufs: int = 3, sizes=None,
             dy_bufs: int = 1, y_first: bool = False,
             z_pool_items=(), final_dve_items=(),
             xprod_pool=(), mbufs: int = 2, dma_act_items=()) -> bass.Bass:
    xh = xs + 2
    nc = bacc.Bacc(None)
    if sizes is None:
        sizes = SIZES if xs == 32 else [(KX, KX)] * (xs // KX)

    uw_h = nc.declare_dram_parameter("uw", [Y, 2, D, xh, ZP], F16, isOutput=False)
    w_h = nc.declare_dram_parameter("wmats", [Y, 4 * Y], F16, isOutput=False)
    w8_h = nc.declare_dram_parameter("w8", [Y, 2, Y], F8, isOutput=False)
    na = len(HOST_AXES)
    hd_h = (nc.declare_dram_parameter("hd", [Y, na, 2, xs, Z], F16,
                                      isOutput=False) if na else None)
    out_h = nc.declare_dram_parameter("out", [D, Y, xs, Z], F16, isOutput=True)

    with tile.TileContext(nc) as tc:
        with (
            tc.tile_pool(name="inp", bufs=1) as inp,
            tc.tile_pool(name="wp", bufs=1) as wp,
            tc.tile_pool(name="dpool", bufs=dbufs) as dpool,
            tc.tile_pool(name="ppool", bufs=pbufs) as ppool,
            tc.tile_pool(name="mpool", bufs=mbufs) as mpool,
            tc.tile_pool(name="psum_dy", bufs=dy_bufs, space="PSUM") as psum_dy,
            tc.tile_pool(name="psum_acc", bufs=accbufs, space="PSUM") as psum_acc,
            tc.tile_pool(name="spool", bufs=sbufs) as spool,
        ):
            wt = wp.tile([Y, 4 * Y], F16, name="wt")
            nc.sync.dma_start(out=wt[:, :], in_=w_h[:, :])
            w8 = wp.tile([Y, 2, Y], F8, name="w8")
            nc.sync.dma_start(out=w8[:, :, :], in_=w8_h[:, :, :])
            dyT = wt[:, 0:Y]
            ndyT = wt[:, Y : 2 * Y]
            eyeT = wt[:, 2 * Y : 3 * Y]
            eye8T = wt[:, 3 * Y : 4 * Y]

            # Load the combined uw tensor in x-splits so early chunks can
            # start while the rest streams in (Tile tracks subtile deps).
            # Each split boundary is the halo-extended need of one work
            # item.  The first split is issued per side (half the bytes) so
            # the first item's diffs can start as early as possible.
            cuts = []
            off0 = 0
            for kb, _ in sizes:
                need = min(xh, off0 + kb + 2)
                if not cuts or need > cuts[-1]:
                    cuts.append(need)
                off0 += kb
            uw = inp.tile([Y, 2, D, xh, ZP], F16, name="uw", tag="uw")
            dma_splits = []
            for s in range(2):
                nc.sync.dma_start(out=uw[:, s, :, 0 : cuts[0], :],
                                  in_=uw_h[:, s, :, 0 : cuts[0], :])
                dma_splits.append((s, 0))
            hd = inp.tile([Y, na, 2, xs, Z], F16, name="hd", tag="hd") \
                if na else None
            if na:
                a0 = min(cuts[0] - 2, xs)
                nc.sync.dma_start(out=hd[:, :, :, 0:a0, :],
                                  in_=hd_h[:, :, :, 0:a0, :])
            for a, b2 in zip(cuts, cuts[1:]):
                nc.sync.dma_start(out=uw[:, :, :, a:b2, :],
                                  in_=uw_h[:, :, :, a:b2, :])
                dma_splits.append((0, a))
                if na:
                    ha, hb = min(a - 2, xs), min(b2 - 2, xs)
                    if hb > ha:
                        nc.sync.dma_start(out=hd[:, :, :, ha:hb, :],
                                          in_=hd_h[:, :, :, ha:hb, :])

            zc = slice(2, 2 + Z)       # center z view
            zp1 = slice(3, 3 + Z)      # z+1
            zm1 = slice(1, 1 + Z)      # z-1

            # Prime PE's vector clock against every input DMA with tiny
            # matmuls, so real matmuls never need a second (DMA) wait —
            # TRN2 matmul instructions support a single sync wait.
            scratch = psum_acc.tile([8, 8], F32, name="scratch", tag="acc")
            for s, a in dma_splits:
                nc.tensor.matmul(scratch[:, 0:1], wt[:, 0:8],
                                 uw[:, s, 0, a : a + 1, 0:1],
                                 start=True, stop=True)
            nc.tensor.matmul(scratch[:, 0:1], w8[:, 0, 0:8],
                             uw[:, 0, 0, 0:1, 0:1], start=True, stop=True)

            items = []
            off = 0
            for sz, kx in sizes:
                items.append((off, sz, kx))
                off += sz
            assert off == xs
            final_set = {i % len(items) for i in final_dve_items}
            dma_act_set = {i % len(items) for i in dma_act_items}

            def stage_a(idx, item):
                """diffs + products for work item (x0, kb planes)."""
                x0, kb, kx = item
                tail_item = idx >= len(sizes) - 1
                u0 = 1 + x0
                KB = kb
                xsl = slice(u0, u0 + KB)
                xp1 = slice(u0 + 1, u0 + 1 + KB)
                xm1 = slice(u0 - 1, u0 - 1 + KB)
                ng = max(1, KB // (2 * kx))   # 8-plane (or kb) final groups
                gw = KB // ng                 # planes per group
                nh = gw // kx                 # kx-chunks per group

                # Batched diffs, one op per (axis, slot) across all 3
                # channels: slot0 = D w (all ch), slot1 = -D u (reversed).
                # z-diffs of early items run on GPSIMD: it is otherwise idle
                # until the first evac'd y-chunk arrives, and this trims the
                # saturated DVE region.
                zeng = nc.gpsimd if idx in z_pool_items else nc.vector
                # channel-0 diffs of host-offloaded axes come from hd, so
                # the on-device batched diff ops (and tiles) skip channel 0
                # there: the tile then holds channels 1..2 at indices 0..1.
                hx, hz = "x" in HOST_AXES, "z" in HOST_AXES
                cx = slice(1, D) if hx else slice(0, D)
                cz = slice(1, D) if hz else slice(0, D)
                nx, nz = D - int(hx), D - int(hz)
                px = dpool.tile([Y, 2, nx, KB, Z], F16, name="px", tag="px")
                nc.vector.tensor_sub(out=px[:, 0, :, :, :],
                                     in0=uw[:, 1, cx, xp1, zc],
                                     in1=uw[:, 1, cx, xm1, zc])
                nc.vector.tensor_sub(out=px[:, 1, :, :, :],
                                     in0=uw[:, 0, cx, xm1, zc],
                                     in1=uw[:, 0, cx, xp1, zc])
                pz = dpool.tile([Y, 2, nz, KB, Z], F16, name="pz", tag="pz")
                zeng.tensor_sub(out=pz[:, 0, :, :, :],
                                in0=uw[:, 1, cz, xsl, zp1],
                                in1=uw[:, 1, cz, xsl, zm1])
                zeng.tensor_sub(out=pz[:, 1, :, :, :],
                                in0=uw[:, 0, cz, xsl, zm1],
                                in1=uw[:, 0, cz, xsl, zp1])
                xsl0 = slice(x0, x0 + KB)   # hd is center-indexed (no halo)

                chunk = []
                for i in range(D):
                    wv = uw[:, 1, i]     # w field, [Y, xh, ZP] view
                    uv = uw[:, 0, i]     # u field

                    def do_y():
                        # y path per final group: matmul diffs -> psum, one
                        # Act evac to fp16, GPSIMD product pairs (fp8)
                        ppys = []
                        for g in range(ng):
                            ylr = psum_dy.tile([Y, nh, 2, kx, Z], F32,
                                               name="ylr", tag="ylr")
                            for h in range(nh):
                                hs = slice(u0 + g * gw + h * kx,
                                           u0 + g * gw + h * kx + kx)
                                nc.tensor.matmul(ylr[:, h, 0, :, :], dyT,
                                                 wv[:, hs, zc],
                                                 start=True, stop=True)
                                nc.tensor.matmul(ylr[:, h, 1, :, :], ndyT,
                                                 uv[:, hs, zc],
                                                 start=True, stop=True)
                            dylr = dpool.tile([Y, nh, 2, kx, Z], F16,
                                              name="dylr", tag="dylr")
                            nc.scalar.copy(
                                out=dylr[:, :, :, :, :]
                                    .rearrange("p a b c d -> p (a b c d)"),
                                in_=ylr[:, :, :, :, :]
                                    .rearrange("p a b c d -> p (a b c d)"))
                            for h in range(nh):
                                hs = slice(u0 + g * gw + h * kx,
                                           u0 + g * gw + h * kx + kx)
                                if tail_item:
                                    ppy = ppool.tile([Y, 2, kx, Z], F16,
                                                     name="ppy", tag="ppy")
                                    nc.vector.tensor_mul(
                                        out=ppy[:, :, :, :],
                                        in0=dylr[:, h, :, :, :],
                                        in1=uw[:, :, 1, hs, zc])
                                else:
                                    ppy = ppool.tile([Y, 2, kx, Z], F8,
                                                     name="ppy", tag="ppy")
                                    nc.gpsimd.tensor_mul(
                                        out=ppy[:, :, :, :],
                                        in0=dylr[:, h, :, :, :],
                                        in1=uw[:, :, 1, hs, zc])
                                ppys.append(ppy)
                        return ppys

                    def do_xz():
                        # x-product pair on DVE (fp16 keeps the 2x mode):
                        # slot0 = Dxw_i * u_0, slot1 = -Dxu_i * w_0
                        ppx = ppool.tile([Y, 2, KB, Z], F16, name="ppx",
                                         tag="ppx")
                        nc.vector.tensor_mul(out=ppx[:, :, :, :],
                                             in0=px[:, :, i, :, :],
                                             in1=uw[:, :, 0, xsl, zc])
                        # z-product pair: GPSIMD fp8 (DoubleRow fodder) in
                        # steady state; the tail item runs on DVE in fp16
                        # (Pool drains last, DVE is idle by then).
                        if tail_item:
                            ppz = ppool.tile([Y, 2, KB, Z], F16, name="ppz",
                                             tag="ppz")
                            nc.vector.tensor_mul(out=ppz[:, :, :, :],
                                                 in0=pz[:, :, i, :, :],
                                                 in1=uw[:, :, 2, xsl, zc])
                        else:
                            ppz = ppool.tile([Y, 2, KB, Z], F8, name="ppz",
                                             tag="ppz")
                            nc.gpsimd.tensor_mul(out=ppz[:, :, :, :],
                                                 in0=pz[:, :, i, :, :],
                                                 in1=uw[:, :, 2, xsl, zc])
                        return ppx, ppz

                    if y_first:
                        ppys = do_y()
                        ppx, ppz = do_xz()
                    else:
                        ppx, ppz = do_xz()
                        ppys = do_y()
                    chunk.append((ppx, ppz, ppys, tail_item, False))
                return chunk

            def stage_b(b_idx, item, chunk):
                """PSUM accumulation + final scale + DMA out (per group)."""
                x0i, kb, kx = item
                final_dve = b_idx in final_set
                u0 = 1 + x0i
                ng = max(1, kb // (2 * kx))
                gw = kb // ng
                nh = gw // kx
                for i in range(D):
                    ppx, ppz, ppys, tail_item, ppx_f8 = chunk[i]
                    for g in range(ng):
                        acc = psum_acc.tile([Y, nh, kx * Z], F32, name="acc",
                                            tag="acc")
                        for h in range(nh):
                            lo = g * gw + h * kx
                            hs = slice(lo, lo + kx)
                            av = acc[:, h, :].rearrange(
                                "p (a b) -> p a b", a=kx)
                            # 8*u inject first: start matmul carries only the
                            # PSUM-slot WAR wait.
                            nc.tensor.matmul(
                                av, eye8T,
                                uw[:, 0, i, u0 + lo : u0 + lo + kx, zc],
                                start=True, stop=False)
                            if ppx_f8:
                                nc.tensor.matmul(av, w8[:, :, :],
                                                 ppx[:, :, hs, :],
                                                 start=False, stop=False,
                                                 perf_mode=DR)
                            else:
                                nc.tensor.matmul(av, eyeT, ppx[:, 0, hs, :],
                                                 start=False, stop=False)
                                nc.tensor.matmul(av, eyeT, ppx[:, 1, hs, :],
                                                 start=False, stop=False)
                            ppy = ppys[g * nh + h]
                            if tail_item:
                                nc.tensor.matmul(av, eyeT, ppy[:, 0, :, :],
                                                 start=False, stop=False)
                                nc.tensor.matmul(av, eyeT, ppy[:, 1, :, :],
                                                 start=False, stop=False)
                                nc.tensor.matmul(av, eyeT, ppz[:, 0, hs, :],
                                                 start=False, stop=False)
                                nc.tensor.matmul(av, eyeT, ppz[:, 1, hs, :],
                                                 start=False, stop=True)
                            else:
                                nc.tensor.matmul(av, w8[:, :, :],
                                                 ppy[:, :, :, :],
                                                 start=False, stop=False,
                                                 perf_mode=DR)
                                nc.tensor.matmul(av, w8[:, :, :],
                                                 ppz[:, :, hs, :],
                                                 start=False, stop=True,
                                                 perf_mode=DR)
                        stage = spool.tile([Y, gw, Z], F16, name="stage",
                                           tag="stage")
                        if final_dve:
                            # last items: DVE is idle by now, and this skips
                            # the Act queue's end-of-kernel evac backlog
                            nc.vector.tensor_scalar_mul(
                                stage[:, :, :].rearrange("p a b -> p (a b)"),
                                acc[:, :, :].rearrange("p a b -> p (a b)"),
                                0.125)
                        else:
                            nc.scalar.mul(
                                stage[:, :, :].rearrange("p a b -> p (a b)"),
                                acc[:, :, :].rearrange("p a b -> p (a b)"),
                                0.125)
                        x0 = x0i + g * gw
                        # tail items: fan the per-channel out-DMAs across
                        # three queues so the final drain isn't serialized
                        # on SP (~790 ns each at the very end of the kernel)
                        if b_idx in dma_act_set:
                            dq = (nc.sync, nc.scalar, nc.gpsimd)[i % 3]
                        else:
                            dq = nc.sync
                        dq.dma_start(
                            out=out_h[i, :, x0 : x0 + gw, :],
                            in_=stage[:, :, :])

            # software pipeline: A(0), A(1), B(0), A(2), B(1), ... B(last)
            prev = None
            prev_chunk = None
            for idx, item in enumerate(items):
                ch = stage_a(idx, item)
                if prev is not None:
                    stage_b(idx - 1, prev, prev_chunk)
                prev, prev_chunk = item, ch
            stage_b(len(items) - 1, prev, prev_chunk)

    if not nc.is_finalized():
        nc.finalize()
    return nc


def _host_shard(u_b: np.ndarray, w_b: np.ndarray, xs: int) -> list[np.ndarray]:
    """(D, X, Y, Z) f32 pair -> list over x-slabs of (Y, 2, D, xs+2, ZP) fp16."""
    slabs = []
    for s in range(X // xs):
        idx = (np.arange(-1, xs + 1) + s * xs) % X
        su = u_b[:, idx, :, :]                    # (D, xs+2, Y, Z)
        sw = w_b[:, idx, :, :]
        sl = np.stack([su, sw], axis=0)           # (2, D, xs+2, Y, Z)
        sl = np.transpose(sl, (3, 0, 1, 2, 4))    # (Y, 2, D, xs+2, Z)
        sl = np.concatenate([sl[..., 126:128], sl, sl[..., 0:2]], axis=-1)
        slabs.append(np.ascontiguousarray(sl.astype(np.float16)))
    return slabs


def _host_diffs(u_b: np.ndarray, w_b: np.ndarray, xs: int) -> list[np.ndarray]:
    """Channel-0 diff pairs per x-slab: (Y, naxes, 2, xs, Z) fp16.

    slot0 = D(w_0), slot1 = -D(u_0) for each axis in HOST_AXES, matching the
    sign fold of the on-device diffs (circulant wrap via full-X roll).
    """
    axes = []
    for ax in HOST_AXES:
        a = 0 if ax == "x" else 2   # axis within (X, Y, Z) of channel slice
        dw = np.roll(w_b[0], -1, axis=a) - np.roll(w_b[0], 1, axis=a)
        du = np.roll(u_b[0], 1, axis=a) - np.roll(u_b[0], -1, axis=a)
        axes.append(np.stack([dw, du], axis=0))   # (2, X, Y, Z)
    full = np.stack(axes, axis=0)                 # (na, 2, X, Y, Z)
    full = np.transpose(full, (3, 0, 1, 2, 4))    # (Y, na, 2, X, Z)
    return [np.ascontiguousarray(full[:, :, :, s * xs:(s + 1) * xs, :]
                                 .astype(np.float16))
            for s in range(X // xs)]


def kernel(left: np.ndarray, right: np.ndarray) -> np.ndarray:
    left = np.asarray(left)
    right = np.asarray(right)
    assert left.shape == (B, D, X, Y, Z), left.shape

    wmats = _make_wmats()
    w8 = _make_w8()
    slabs_per_batch = X // XS  # 4

    lf = np.asarray(left, dtype=np.float32)
    rf = np.asarray(right, dtype=np.float32)
    shards = [_host_shard(lf[b] + rf[b], lf[b] - rf[b], XS) for b in range(B)]
    hds = [_host_diffs(lf[b] + rf[b], lf[b] - rf[b], XS) for b in range(B)] \
        if HOST_AXES else None

    maps = []
    for core in range(NCORES):
        b, s = divmod(core, slabs_per_batch)
        m = {
            "uw": shards[b][s],
            "wmats": wmats,
            "w8": w8,
        }
        if HOST_AXES:
            m["hd"] = hds[b][s]
        maps.append(m)

    nc = build_nc(XS)
    res = run_bass_kernel_spmd(nc, maps, core_ids=list(range(NCORES)))

    out = np.empty((B, D, X, Y, Z), dtype=np.float32)
    for core in range(NCORES):
        b, s = divmod(core, slabs_per_batch)
        o = res.results[core]["out"].astype(np.float32)   # (D, Y, XS, Z)
        out[b, :, s * XS : (s + 1) * XS, :, :] = np.transpose(o, (0, 2, 1, 3))
    return out


# ---------------------------------------------------------------------------
# numpy reference of the same math (for probing without jax)
def _np_ref(left: np.ndarray, right: np.ndarray) -> np.ndarray:
    l = np.moveaxis(left, 1, -1).astype(np.float64)
    r = np.moveaxis(right, 1, -1).astype(np.float64)

    def jac(v):
        cols = []
        for j in range(3):
            ax = 1 + j
            g = (np.roll(v, -1, axis=ax) - np.roll(v, 1, axis=ax)) * 0.5
            cols.append(g)
        return np.stack(cols, axis=-1)

    jx, jy = jac(l), jac(r)
    br = np.einsum("bxyzij,bxyzj->bxyzi", jx, r) - np.einsum(
        "bxyzij,bxyzj->bxyzi", jy, l)
    z = l + r + 0.5 * br
    return np.moveaxis(z, -1, 1).astype(np.float32)


if __name__ == "__main__":
    import os
    probe_xs = int(os.environ.get("PROBE_XS", "8"))
    probe_cores = int(os.environ.get("PROBE_CORES", "1"))
    rng = np.random.default_rng(0)
    lf = rng.standard_normal((1, D, X, Y, Z), dtype=np.float32)
    rf = rng.standard_normal((1, D, X, Y, Z), dtype=np.float32)

    sh = _host_shard(lf[0] + rf[0], lf[0] - rf[0], probe_xs)
    hdsh = _host_diffs(lf[0] + rf[0], lf[0] - rf[0], probe_xs) \
        if HOST_AXES else None
    wm = _make_wmats()
    w8m = _make_w8()
    maps = []
    for c in range(probe_cores):
        m = {"uw": sh[c], "wmats": wm, "w8": w8m}
        if HOST_AXES:
            m["hd"] = hdsh[c]
        maps.append(m)

    import time
    t0 = time.time()
    nc = build_nc(probe_xs)
    t1 = time.time()
    print(f"build: {t1-t0:.1f}s", flush=True)
    res = run_bass_kernel_spmd(nc, maps, core_ids=list(range(probe_cores)))
    t2 = time.time()
    print(f"compile+run: {t2-t1:.1f}s", flush=True)

    ref = _np_ref(lf, rf)
    for c in range(probe_cores):
        o = res.results[c]["out"].astype(np.float32)   # (D, Y, xs, Z)
        o = np.transpose(o, (0, 2, 1, 3))         # (D, xs, Y, Z)
        expect = ref[0, :, c * probe_xs : (c + 1) * probe_xs]
        err = np.abs(o - expect)
        rel = np.linalg.norm(o - expect) / np.linalg.norm(expect)
        print(f"core {c}: rel={rel:.3e} absmax={err.max():.3e} "
              f"out_absmax={np.abs(expect).max():.3f}")
